# revision 43
# baseline (speedup 1.0000x reference)
"""CQVAE loss kernel for Trainium2, data-parallel over batch on 8 NeuronCores.

loss = kld(qy) + mse(gather(rzs), zs[:, :Sg]) + bias(best, best_gt)
       + bias(gather(pts), gts)
where bias(p, g) = mse(p, g) + 10 * mse(p[..., MARK, :], g[..., MARK, :]).

Each core handles 16 of the 128 batches.  The four big streams (zs, rzs,
pts, gts) ride three SWDGE queues as multi-index dma_gather instructions
(zs/gts with sequential/permutation indices) — the software-DGE path emits
large per-engine bursts and sidesteps the ~20-packet/us per-queue
descriptor rate of the HW DGE, which only carries the small qy/best loads.
Gather calls stay <=512 indices (half the per-queue descriptor ring) so
ring flow-control pipelines them.  zs/rzs/pts/gts are cast to bf16 on the
host (loss tolerance is 2e-2; measured error ~1e-4), halving HBM traffic;
qy stays f32 for the KLD log path.  pts/gts rows are host-padded 236 -> 256
values (gather rows must be 256B multiples).  Squared-difference sums are
reduced per partition on the vector/scalar engines; each core ships a
[128, 32] stats tile and the host folds it.
"""

import sys

import numpy as np

try:
    import concourse  # noqa: F401
except ImportError:  # pragma: no cover
    sys.path.insert(0, "/opt/trn_rl_repo")

import concourse.bass as bass
import concourse.mybir as mybir
import concourse.tile as tile
from concourse import bacc, library_config
from concourse.bass_utils import run_bass_kernel_spmd

F32 = mybir.dt.float32
BF16 = mybir.dt.bfloat16
I16 = mybir.dt.int16
AX = mybir.AxisListType
OP = mybir.AluOpType
ACTF = mybir.ActivationFunctionType

NCORES = 8
B, S, SG, D, P, V = 128, 256, 128, 1024, 118, 64
BL = B // NCORES  # batches per core
P2 = 2 * P  # 236 values per point-row
PE = 256  # padded point-row length (gather elem_size must be 256B-multiple)
MARK = (0, 29, 88, 117)
EPS = 1e-20
ALPHA = 10.0

K = 16  # point-pair blocks per partition (= BL)
# ae pieces: batch ranges, <=4 batches (=512 idxs) per dma_gather
AE_PIECES = [(0, 4), (4, 8), (8, 12), (12, 14), (14, 16)]
# pts/gts gather pieces: k ranges
PT_PIECES = [(0, 4), (4, 8), (8, 12), (12, 16)]
QN = BL * S // 128  # 32 qy rows per partition
QCOLS = QN * V  # 2048
NSTAT = 32
WR, WP, WZ, WG = 128, 128, 128, 128  # idx tile col ranges per stream

DT = BF16  # on-device dtype of the big streamed tensors

_module = None
last_results = None  # BassKernelResults of the most recent run (for profiling)


def _build_module():
    nc = bacc.Bacc(num_swdge_queues=3)

    zs = nc.dram_tensor("zs", [BL * S, D], DT, kind="ExternalInput")
    rzs = nc.dram_tensor("rzs", [BL * S, D], DT, kind="ExternalInput")
    pts = nc.dram_tensor("pts", [BL * S, PE], DT, kind="ExternalInput")
    gts = nc.dram_tensor("gts", [BL * SG, PE], DT, kind="ExternalInput")
    qy = nc.dram_tensor("qy", [BL * S, V], F32, kind="ExternalInput")
    best = nc.dram_tensor("best", [BL, P2], F32, kind="ExternalInput")
    best_gt = nc.dram_tensor("best_gt", [BL, P2], F32, kind="ExternalInput")
    # i16 gather indices, [16, W] wrapped + replicated x8.  idxa columns:
    # 0..127   rzs: flat t = b*128 + i   -> row b*S + mapping[b, i]
    # 128..255 pts: flat t = k*128 + p   -> row of pair (16p + k)
    idxa = nc.dram_tensor("idxa", [128, WR + WP], I16, kind="ExternalInput")
    out = nc.dram_tensor("out", [128, NSTAT], F32, kind="ExternalOutput")

    qy_v = qy[:].rearrange("(p n) v -> p (n v)", n=QN)
    gts_v = gts[:].rearrange("(p k) c -> p (k c)", k=K)
    zs_v = zs[:].rearrange("(b s) d -> s b d", s=S)  # [S, BL, D]

    with tile.TileContext(nc) as tc:
        with tc.tile_pool(name="main", bufs=1) as pool:
            # --- tiles ---
            idxa_t = pool.tile([128, WR + WP], I16)
            stats = pool.tile([128, NSTAT], F32)
            ebias = pool.tile([128, 1], F32)
            qy_t = pool.tile([128, QCOLS], F32)
            lg = pool.tile([128, QCOLS], F32)
            gt = pool.tile([128, K * PE], DT)
            pg = pool.tile([128, K * PE], DT)
            zt = [
                pool.tile([128, (b1 - b0) * D], DT, name=f"zt{i}")
                for i, (b0, b1) in enumerate(AE_PIECES)
            ]
            rg = [
                pool.tile([128, (b1 - b0) * D], DT, name=f"rg{i}")
                for i, (b0, b1) in enumerate(AE_PIECES)
            ]
            bt = pool.tile([BL, P2], F32)
            bgt = pool.tile([BL, P2], F32)
            bm4 = pool.tile([BL, 4], F32)

            # --- index load leads the scalar HW queue; gts (pair layout,
            # 8KB-contiguous per partition) on sync; best, qy on scalar ---
            nc.scalar.dma_start(idxa_t[:], idxa[:])
            nc.sync.dma_start(gt[:], gts_v)
            nc.scalar.dma_start(bt[:], best[:])
            nc.scalar.dma_start(bgt[:], best_gt[:])
            nc.scalar.dma_start(qy_t[:], qy_v)

            # --- init ---
            nc.vector.memset(stats[:], 0.0)
            nc.vector.memset(ebias[:], float(V) * EPS)

            # --- the four big streams on three SWDGE queues ---
            # hoisted num_idxs registers (one MOVE each instead of per call)
            ni_reg = {
                ni: nc.gpsimd.to_reg(ni)
                for ni in sorted({(b1 - b0) * 128 for b0, b1 in AE_PIECES}
                                 | {(k1 - k0) * 128 for k0, k1 in PT_PIECES})
            }

            def z_load(i):
                # zs is sequential: mainline SWDGE direct DMA (queue 0, no
                # ucode library needed) — big burst packets from t~8us
                b0, b1 = AE_PIECES[i]
                nc.gpsimd.dma_start(
                    zt[i][:].rearrange("p (k d) -> p k d", d=D),
                    zs_v[0:SG, b0:b1, :],
                )

            def rz_gather(i):
                b0, b1 = AE_PIECES[i]
                ni = (b1 - b0) * 128
                nc.gpsimd.dma_gather(
                    rg[i][:].rearrange("p (k d) -> p k d", d=D),
                    rzs[:],
                    idxa_t[:, b0 * 8 : b1 * 8],
                    ni, ni_reg[ni], D, queue_num=1,
                )

            def pt_gather(j):
                k0, k1 = PT_PIECES[j]
                ni = (k1 - k0) * 128
                nc.gpsimd.dma_gather(
                    pg[:, k0 * PE : k1 * PE].rearrange("p (k c) -> p k c", c=PE),
                    pts[:],
                    idxa_t[:, 128 + k0 * 8 : 128 + k1 * 8],
                    ni, ni_reg[ni], PE, queue_num=2,
                )

            # z0+z1 stream on queue 0 while the gather ucode library loads
            # (~12us Q7 link stall); then the rest, rzs on Q1, pts on Q2.
            z_load(0)
            z_load(1)
            nc.gpsimd.load_library(library_config.mlp)
            rz_gather(0)
            pt_gather(0)
            z_load(2)
            rz_gather(1)
            pt_gather(1)
            z_load(3)
            rz_gather(2)
            pt_gather(2)
            z_load(4)
            rz_gather(3)
            pt_gather(3)
            rz_gather(4)

            # --- KLD: sum q * (log(q + eps) - log(1/V)) via log(V*q + V*eps) ---
            nc.scalar.activation(lg[:], qy_t[:], ACTF.Ln, bias=ebias[:], scale=float(V))
            nc.vector.scalar_tensor_tensor(
                out=lg[:],
                in0=lg[:],
                scalar=0.0,
                in1=qy_t[:],
                op0=OP.subtract,
                op1=OP.mult,
                accum_out=stats[:, 12:13],
            )

            # --- BEST: per-core shard [BL, P2] ---
            nc.vector.tensor_sub(bt[:], bt[:], bgt[:])
            nc.scalar.activation(
                bt[:], bt[:], ACTF.Square, accum_out=stats[:BL, 13:14]
            )

            # --- BIAS pieces: (pts_g - gts)^2 -> stats cols 8..11 ---
            def bias_piece(j):
                k0, k1 = PT_PIECES[j]
                sl = slice(k0 * PE, k1 * PE)
                nc.vector.tensor_sub(pg[:, sl], pg[:, sl], gt[:, sl])
                nc.scalar.activation(
                    pg[:, sl], pg[:, sl], ACTF.Square,
                    accum_out=stats[:, 8 + j : 9 + j],
                )

            # --- AE pieces: (rzs_g - zs)^2 -> stats cols 0..4 ---
            def ae_piece(i):
                nc.vector.tensor_sub(rg[i][:], rg[i][:], zt[i][:])
                nc.scalar.activation(
                    rg[i][:], rg[i][:], ACTF.Square,
                    accum_out=stats[:, i : i + 1],
                )



            ae_piece(0)
            bias_piece(0)

            # best-mark folds (tiny), after best square
            for j, m in enumerate(MARK):
                nc.vector.reduce_sum(
                    out=bm4[:, j : j + 1], in_=bt[:, 2 * m : 2 * m + 2], axis=AX.X
                )
            nc.vector.reduce_sum(out=stats[:BL, 14:15], in_=bm4[:], axis=AX.X)

            ae_piece(1)
            bias_piece(1)
            ae_piece(2)
            bias_piece(2)
            bias_piece(3)

            # bias-mark folds, after all bias squares
            pg3 = pg[:].rearrange("p (k c) -> p k c", c=PE)
            for j, m in enumerate(MARK):
                nc.vector.reduce_sum(
                    out=stats[:, 16 + j : 17 + j],
                    in_=pg3[:, :, 2 * m : 2 * m + 2],
                    axis=AX.XY,
                )

            ae_piece(3)
            ae_piece(4)

            # ship per-partition stats; the host folds the 128 partitions
            nc.sync.dma_start(out[:], stats[:])

    nc.compile()
    return nc


def _to_dev_dtype(a):
    if DT == F32:
        return np.ascontiguousarray(a, dtype=np.float32)
    import ml_dtypes

    return np.ascontiguousarray(a.astype(ml_dtypes.bfloat16))


def _i16_tile(flat):
    """[16, len/16] i16 gather-index tile (t at [t%16, t//16]), replicated x8."""
    w = flat.shape[0] // 16
    return np.tile(flat.reshape(w, 16).T, (8, 1)).astype(np.int16)


def kernel(
    zs, rzs, pts, best, qy, gts, best_gt, mapping, vector_dims, **trace_kwargs
):
    global _module, last_results
    vd = int(np.asarray(vector_dims))
    assert vd == V, f"kernel compiled for vector_dims={V}, got {vd}"

    if _module is None:
        _module = _build_module()

    zs = np.asarray(zs, dtype=np.float32)
    rzs = np.asarray(rzs, dtype=np.float32)
    pts = np.asarray(pts, dtype=np.float32)
    gts = np.asarray(gts, dtype=np.float32)
    qy = np.asarray(qy, dtype=np.float32)
    mapping = np.asarray(mapping).astype(np.int32)
    best2 = np.ascontiguousarray(np.asarray(best, dtype=np.float32).reshape(B, P2))
    bgt2 = np.ascontiguousarray(np.asarray(best_gt, dtype=np.float32).reshape(B, P2))

    # pad point rows 236 -> 256 values
    pts_pad = np.zeros((B, S, PE), np.float32)
    pts_pad[:, :, :P2] = pts.reshape(B, S, P2)
    gts_pad = np.zeros((B, SG, PE), np.float32)
    gts_pad[:, :, :P2] = gts.reshape(B, SG, P2)

    base = (np.arange(BL, dtype=np.int32) * S)[:, None]
    # pts flat order: t = k*128 + p -> pair = gts row 16p + k -> (b, i)
    pair = 16 * np.arange(128)[None, :] + np.arange(K)[:, None]  # [k, p]
    pbb, pii = pair // SG, pair % SG
    in_maps = []
    for c in range(NCORES):
        sl = slice(c * BL, (c + 1) * BL)
        rows = mapping[sl] + base  # [BL, SG] = b*S + mapping[b, i]
        idxa_np = np.hstack(
            [_i16_tile(rows.ravel()), _i16_tile(rows[pbb, pii].ravel())]
        )
        in_maps.append(
            {
                "zs": _to_dev_dtype(zs[sl].reshape(BL * S, D)),
                "rzs": _to_dev_dtype(rzs[sl].reshape(BL * S, D)),
                "pts": _to_dev_dtype(pts_pad[sl].reshape(BL * S, PE)),
                "gts": _to_dev_dtype(gts_pad[sl].reshape(BL * SG, PE)),
                "qy": np.ascontiguousarray(qy[sl].reshape(BL * S, V)),
                "best": np.ascontiguousarray(best2[sl]),
                "best_gt": np.ascontiguousarray(bgt2[sl]),
                "idxa": np.ascontiguousarray(idxa_np),
            }
        )

    last_results = run_bass_kernel_spmd(
        _module, in_maps, list(range(NCORES)), **trace_kwargs
    )
    tot = (
        np.stack(
            [
                np.asarray(r["out"], dtype=np.float64).reshape(128, NSTAT).sum(axis=0)
                for r in last_results.results
            ]
        )
        .sum(axis=0)
    )

    ae_loss = tot[0:5].sum() / (B * SG * D)
    bias_loss = tot[8:12].sum() / (B * SG * P2) + ALPHA * tot[16:20].sum() / (
        B * SG * 2 * len(MARK)
    )
    kld_loss = tot[12] / (B * S)
    best_mse = tot[13] / (B * P2) + ALPHA * tot[14] / (B * 2 * len(MARK))

    return np.array(kld_loss + ae_loss + best_mse + bias_loss, dtype=np.float32)
